# revision 12
# baseline (speedup 1.0000x reference)
"""GAT 2-layer kernel for 8 Trainium2 NeuronCores.

Sharding: destination nodes split into 8 contiguous chunks of 2500; each core
owns the segment softmax + weighted scatter for its chunk. Extended feature
tables (h1ext = [h1(512)|aS1(4)|pad], h2ext = [h2(64)|aS2(1)|pad]) are
exchanged via HBM AllGather; per-edge source rows are fetched with dma_gather
(one descriptor per edge); segment softmax + scatter-add are selection
matmuls on the PE (lhsT = 0/1 dst-selection matrix built from iota compares).
float32r (TF32-like) is used on the PE for 1 cycle/row; selection matrices
are exact 0/1 so only the feature operand sees the ~1e-4 rounding.
Softmax skips the max-subtraction (score range ~[-1, 4.1]; exp is safe and
mathematically identical).
"""
import os
import sys
import types

sys.path.insert(0, "/opt/trn_rl_repo")
sys.path.insert(0, "/root/.axon_site")

import numpy as np

try:
    from trn_agent_boot.trn_boot import _ntff_profile_via_ctypes
    _hook = _ntff_profile_via_ctypes('/opt/axon/libaxon_pjrt.so')
    _m = types.ModuleType('antenv.axon_hooks')
    _m.get_axon_ntff_profile_hook = lambda: _hook
    sys.modules['antenv.axon_hooks'] = _m
except Exception:
    pass

import concourse.bacc as bacc
import concourse.mybir as mybir
from concourse.bass_utils import run_bass_kernel_spmd
from concourse.tile import TileContext
from concourse import library_config

f32 = mybir.dt.float32
f32r = mybir.dt.float32r
bf16 = mybir.dt.bfloat16
i16 = mybir.dt.int16
AF = mybir.ActivationFunctionType
ALU = mybir.AluOpType
AX = mybir.AxisListType

N = 20000
IN_DIM = 512
HID = 128
HEADS = 4
D1 = HEADS * HID          # 512
OUT_DIM = 64
NEG_SLOPE = 0.2
NCORES = 8
NPC = N // NCORES         # 2500
NW = (NPC + 127) // 128   # 20
EXT1 = 576                # 2304B rows
EXT2 = 128                # 512B rows

LAST_EXEC_NS = None
_CACHE = {}


def _wrap_idx16(flat):
    K = len(flat)
    out = np.zeros((16, K // 16), np.int16)
    out[np.arange(K) % 16, np.arange(K) // 16] = flat.astype(np.int16)
    return np.tile(out, (8, 1))


def _preprocess(edge_index):
    src = np.concatenate([np.asarray(edge_index[0]), np.arange(N)]).astype(np.int64)
    dst = np.concatenate([np.asarray(edge_index[1]), np.arange(N)]).astype(np.int64)

    core = dst // NPC
    dloc = dst % NPC
    win = dloc // 128
    dwin = dloc % 128

    order = np.lexsort((src, win, core))
    cs, ws = core[order], win[order]
    srt_src, srt_dwin = src[order], dwin[order]
    key = cs * NW + ws
    uniq, starts = np.unique(key, return_index=True)
    ends = np.r_[starts[1:], len(key)]
    buckets = {}
    cnt = np.zeros((NCORES, NW), np.int64)
    for u, s, e in zip(uniq, starts, ends):
        c, w = divmod(int(u), NW)
        buckets[(c, w)] = (srt_src[s:e], srt_dwin[s:e])
        cnt[c, w] = e - s

    T = [max(1, int(np.ceil(cnt[:, w].max() / 128.0))) for w in range(NW)]
    tot = sum(T)

    idx_np = np.zeros((NCORES, 128, tot * 8), np.int16)
    dstcol_np = np.full((NCORES, 128, tot), -1.0, np.float32)
    dstrow_np = np.full((NCORES, 1, tot * 128), -1.0, np.float32)

    for c in range(NCORES):
        off = 0
        for w in range(NW):
            K = T[w] * 128
            s, d = buckets.get((c, w), (np.zeros(0, np.int64), np.zeros(0, np.int64)))
            spad = np.zeros(K, np.int64)
            spad[:len(s)] = s
            dpad = np.full(K, -1.0, np.float32)
            dpad[:len(d)] = d.astype(np.float32)
            idx_np[c, :, off * 8:(off + T[w]) * 8] = _wrap_idx16(spad)
            dstcol_np[c, :, off:off + T[w]] = dpad.reshape(T[w], 128).T
            dstrow_np[c, 0, off * 128:(off + T[w]) * 128] = dpad
            off += T[w]
    return T, tot, idx_np, dstcol_np, dstrow_np


def _build(T, tot, debug=False):
    nc = bacc.Bacc("TRN2", target_bir_lowering=False, debug=False,
                   num_devices=NCORES)

    def din(name, shape, dt=f32):
        return nc.dram_tensor(name, shape, dt, kind="ExternalInput").ap()

    xT = din("xT", [IN_DIM, NPC], f32r)
    W1 = din("W1", [IN_DIM, D1], f32r)
    W2 = din("W2", [D1, OUT_DIM], f32r)
    asrc1b = din("asrc1b", [128, D1])
    adst1b = din("adst1b", [128, D1])
    asrc2b = din("asrc2b", [128, OUT_DIM])
    adst2b = din("adst2b", [128, OUT_DIM])
    b1b = din("b1b", [128, D1])
    b2b = din("b2b", [128, OUT_DIM])
    iota_row = din("iota_row", [128, 128])
    iota_col = din("iota_col", [128, 1])
    ones_row = din("ones_row", [1, 128], bf16)
    ident = din("ident", [128, 128])
    idx = din("idx", [128, tot * 8], i16)
    dstcol = din("dstcol", [128, tot])
    dstrow = din("dstrow", [1, tot * 128], bf16)

    out = nc.dram_tensor("out", [NPC, OUT_DIM], f32, kind="ExternalOutput").ap()
    if debug:
        dbg_h1 = nc.dram_tensor("dbg_h1", [NPC, EXT1], f32, kind="ExternalOutput").ap()
        dbg_hr = nc.dram_tensor("dbg_hr", [NPC, D1], f32, kind="ExternalOutput").ap()
        dbg_h2 = nc.dram_tensor("dbg_h2", [NPC, EXT2], f32, kind="ExternalOutput").ap()

    ag1_in = nc.dram_tensor("ag1_in", [NPC, EXT1], f32).ap()
    ag1_full = nc.dram_tensor("ag1_full", [N, EXT1], f32, addr_space="Shared").ap()
    ag2_in = nc.dram_tensor("ag2_in", [NPC, EXT2], f32).ap()
    ag2_full = nc.dram_tensor("ag2_full", [N, EXT2], f32, addr_space="Shared").ap()

    nc.gpsimd.load_library(library_config.mlp)

    woff = np.concatenate([[0], np.cumsum(T)]).astype(int)

    with TileContext(nc) as tc:
        with (
            nc.allow_low_precision(reason="float32r PE operands (TF32-like)"),
            tc.tile_pool(name="consts", bufs=1) as cp,
            tc.tile_pool(name="resident", bufs=1) as rp,
            tc.tile_pool(name="ps_out", bufs=2, space="PSUM") as ps_out,
            tc.tile_pool(name="ps_small", bufs=1, space="PSUM") as ps_small,
            tc.tile_pool(name="ps_bc", bufs=2, space="PSUM") as ps_bc,
        ):
            # ---------- constants ----------
            w2_t = cp.tile([128, 4, OUT_DIM], f32r)
            nc.sync.dma_start(w2_t[:], W2.rearrange("(c k) d -> k c d", k=128))
            asrc2_t = cp.tile([128, OUT_DIM], f32)
            nc.sync.dma_start(asrc2_t[:], asrc2b)
            adst2_t = cp.tile([128, OUT_DIM], f32)
            nc.sync.dma_start(adst2_t[:], adst2b)
            b1_t = cp.tile([128, D1], f32)
            nc.sync.dma_start(b1_t[:], b1b)
            b2_t = cp.tile([128, OUT_DIM], f32)
            nc.sync.dma_start(b2_t[:], b2b)
            io_row = cp.tile([128, 128], f32)
            nc.sync.dma_start(io_row[:], iota_row)
            io_col = cp.tile([128, 1], f32)
            nc.sync.dma_start(io_col[:], iota_col)
            ones_t = cp.tile([1, 128], bf16)
            nc.sync.dma_start(ones_t[:], ones_row)
            id_t = cp.tile([128, 128], f32)
            nc.sync.dma_start(id_t[:], ident)
            idx_t = cp.tile([128, tot * 8], i16)
            nc.sync.dma_start(idx_t[:], idx)
            dstcol_t = cp.tile([128, tot], f32)
            nc.sync.dma_start(dstcol_t[:], dstcol)
            dstrow_t = cp.tile([1, tot * 128], bf16)
            nc.sync.dma_start(dstrow_t[:], dstrow)

            aD1_res = rp.tile([128, NW, HEADS], f32r)
            aD2_res = rp.tile([128, NW, 1], f32r)

            # ---------- phase A ----------
            pA_ctx = tc.tile_pool(name="pA", bufs=3)
            pAc_ctx = tc.tile_pool(name="pAc", bufs=1)
            pA = pA_ctx.__enter__()
            pAc = pAc_ctx.__enter__()
            xt_sb = pAc.tile([128, 4, NPC], f32r)
            nc.sync.dma_start(xt_sb[:], xT.rearrange("(c k) n -> k c n", k=128))
            w1_t = pAc.tile([128, 4, D1], f32r)
            nc.sync.dma_start(w1_t[:], W1.rearrange("(c k) d -> k c d", k=128))
            asrc1_t = pAc.tile([128, D1], f32)
            nc.sync.dma_start(asrc1_t[:], asrc1b)
            adst1_t = pAc.tile([128, D1], f32)
            nc.sync.dma_start(adst1_t[:], adst1b)
            for w in range(NW):
                rows = min(128, NPC - w * 128)
                ps_h1 = ps_out.tile([128, D1], f32, space="PSUM", tag="out1")
                for k in range(4):
                    nc.tensor.matmul(ps_h1[:rows, :],
                                     xt_sb[:, k, w * 128:w * 128 + rows],
                                     w1_t[:, k, :], start=(k == 0), stop=(k == 3))
                h1e = pA.tile([128, EXT1], f32, tag="h1e")
                nc.vector.tensor_copy(h1e[:rows, 0:D1], ps_h1[:rows, :])
                tmp = pA.tile([128, D1], f32, tag="tmpA")
                nc.vector.tensor_tensor(tmp[:rows, :], ps_h1[:rows, :],
                                        asrc1_t[:rows, :], op=ALU.mult)
                for h in range(HEADS):
                    nc.vector.reduce_sum(h1e[:rows, D1 + h:D1 + h + 1],
                                         tmp[:rows, h * HID:(h + 1) * HID], axis=AX.X)
                nc.vector.tensor_tensor(tmp[:rows, :], ps_h1[:rows, :],
                                        adst1_t[:rows, :], op=ALU.mult)
                for h in range(HEADS):
                    nc.vector.reduce_sum(aD1_res[:rows, w, h:h + 1],
                                         tmp[:rows, h * HID:(h + 1) * HID], axis=AX.X)
                nc.sync.dma_start(ag1_in[w * 128:w * 128 + rows, :], h1e[:rows, :])

            pAc_ctx.__exit__(None, None, None)
            pA_ctx.__exit__(None, None, None)
            sw_ctx = tc.tile_pool(name="sweep", bufs=2)
            s01T_ctx = tc.tile_pool(name="s01Tp", bufs=1)
            sm_ctx = tc.tile_pool(name="small", bufs=2)
            sg_ctx = tc.tile_pool(name="selgp", bufs=3)
            sw = sw_ctx.__enter__()
            s01Tp = s01T_ctx.__enter__()
            sm = sm_ctx.__enter__()
            sg = sg_ctx.__enter__()

            # ---------- AllGather 1 ----------
            nc.gpsimd.collective_compute(
                "AllGather", ALU.bypass, replica_groups=[list(range(NCORES))],
                ins=[ag1_in], outs=[ag1_full])

            if debug:
                for w in range(NW):
                    rows = min(128, NPC - w * 128)
                    t_ = sm.tile([128, EXT1], f32, tag="dbg1")
                    nc.sync.dma_start(t_[:rows, :], ag1_in[w * 128:w * 128 + rows, :])
                    nc.sync.dma_start(dbg_h1[w * 128:w * 128 + rows, :], t_[:rows, :])

            def build_s01T(Tw, off):
                """dst broadcast via K=1 bf16 matmul in 512-wide chunks,
                then compare against the per-partition row index."""
                Kw = Tw * 128
                s01T = s01Tp.tile([128, Tw * 128], f32r, tag="S01T")
                pos = 0
                while pos < Kw:
                    n = min(512, Kw - pos)
                    ps_b = ps_bc.tile([128, 512], f32, space="PSUM", tag="bc")
                    nc.tensor.matmul(ps_b[:, :n], ones_t[:],
                                     dstrow_t[:, off * 128 + pos:off * 128 + pos + n],
                                     start=True, stop=True)
                    nc.vector.tensor_tensor(s01T[:, pos:pos + n], ps_b[:, :n],
                                            io_col[:].to_broadcast([128, n]),
                                            op=ALU.is_equal)
                    pos += n
                return s01T

            # ---------- layer 1 sweep + h2 per window ----------
            for w in range(NW):
                Tw = T[w]
                K = Tw * 128
                off = int(woff[w])
                rows = min(128, NPC - w * 128)

                G = sw.tile([128, Tw, EXT1], f32, tag="G")
                nc.gpsimd.dma_gather(G[:], ag1_full, idx_t[:, off * 8:(off + Tw) * 8],
                                     K, K, EXT1, single_packet=False)

                s01T = build_s01T(Tw, off)
                ps_aDe = ps_small.tile([128, Tw * HEADS], f32, space="PSUM", tag="aDe")
                for t in range(Tw):
                    nc.tensor.matmul(ps_aDe[:, t * HEADS:(t + 1) * HEADS],
                                     s01T[:, t * 128:(t + 1) * 128],
                                     aD1_res[:, w, :], start=True, stop=True)

                # scores -> exp
                sc = sw.tile([128, Tw * HEADS], f32, tag="sc")
                nc.vector.tensor_tensor(
                    sc[:].rearrange("p (t h) -> p t h", h=HEADS),
                    G[:, :, D1:D1 + HEADS],
                    ps_aDe[:].rearrange("p (t h) -> p t h", h=HEADS),
                    op=ALU.add)
                nc.scalar.activation(sc[:], sc[:], AF.Prelu, alpha=NEG_SLOPE)
                expw = sw.tile([128, Tw * HEADS], f32r, tag="expw")
                nc.scalar.activation(expw[:], sc[:], AF.Exp)

                # weighted scatter via selection matmuls
                ps_o1 = ps_out.tile([128, D1], f32, space="PSUM", tag="out1")
                ps_den = ps_small.tile([128, HEADS], f32, space="PSUM", tag="den")
                for t in range(Tw):
                    s01 = sg.tile([128, 128], f32r, tag="S01")
                    nc.vector.tensor_tensor(
                        s01[:], io_row[:],
                        dstcol_t[:, off + t:off + t + 1].to_broadcast([128, 128]),
                        op=ALU.is_equal)
                    gp = sg.tile([128, D1], f32r, tag="Gp")
                    for h in range(HEADS):
                        nc.scalar.activation(
                            gp[:, h * HID:(h + 1) * HID],
                            G[:, t, h * HID:(h + 1) * HID], AF.Copy,
                            scale=expw[:, t * HEADS + h:t * HEADS + h + 1].bitcast(f32))
                    nc.tensor.matmul(ps_o1[:], s01[:], gp[:],
                                     start=(t == 0), stop=(t == Tw - 1))
                    nc.tensor.matmul(ps_den[:], s01[:],
                                     expw[:, t * HEADS:(t + 1) * HEADS],
                                     start=(t == 0), stop=(t == Tw - 1))

                # normalize + bias + relu
                inv = sm.tile([128, HEADS], f32, tag="inv")
                nc.vector.reciprocal(inv[:rows, :], ps_den[:rows, :])
                hr_t = sw.tile([128, D1], f32, tag="hr")
                hr = hr_t[:]
                nc.vector.tensor_tensor(
                    hr.rearrange("p (h j) -> p h j", j=HID)[:rows],
                    ps_o1[:rows, :].rearrange("p (h j) -> p h j", j=HID),
                    inv[:rows, :].rearrange("p (h o) -> p h o", o=1)
                        .to_broadcast([rows, HEADS, HID]),
                    op=ALU.mult)
                nc.vector.tensor_tensor(hr[:rows], hr[:rows], b1_t[:rows, :],
                                        op=ALU.add)
                nc.scalar.activation(hr[:rows], hr[:rows], AF.Relu)

                # h2 = hrelu @ W2 via PE transpose
                ps_h2 = ps_small.tile([128, OUT_DIM], f32, space="PSUM", tag="h2")
                for k in range(4):
                    ps_tr = ps_bc.tile([128, 512], f32, space="PSUM", tag="bc")
                    nc.tensor.transpose(ps_tr[:, :rows],
                                        hr[:rows, k * 128:(k + 1) * 128],
                                        id_t[:rows, :rows])
                    hT = sm.tile([128, 128], f32r, tag="hT")
                    nc.vector.tensor_copy(hT[:, :rows], ps_tr[:, :rows])
                    nc.tensor.matmul(ps_h2[:rows, :], hT[:, :rows], w2_t[:, k, :],
                                     start=(k == 0), stop=(k == 3))
                h2e = sm.tile([128, EXT2], f32, tag="h2e")
                nc.vector.tensor_copy(h2e[:rows, 0:OUT_DIM], ps_h2[:rows, :])
                tmp2 = sm.tile([128, OUT_DIM], f32, tag="tmp2")
                nc.vector.tensor_tensor(tmp2[:rows, :], ps_h2[:rows, :],
                                        asrc2_t[:rows, :], op=ALU.mult)
                nc.vector.reduce_sum(h2e[:rows, OUT_DIM:OUT_DIM + 1],
                                     tmp2[:rows, :], axis=AX.X)
                nc.vector.tensor_tensor(tmp2[:rows, :], ps_h2[:rows, :],
                                        adst2_t[:rows, :], op=ALU.mult)
                nc.vector.reduce_sum(aD2_res[:rows, w, :], tmp2[:rows, :], axis=AX.X)
                nc.sync.dma_start(ag2_in[w * 128:w * 128 + rows, :], h2e[:rows, :])

                if debug:
                    nc.sync.dma_start(dbg_hr[w * 128:w * 128 + rows, :], hr[:rows])
                    nc.sync.dma_start(dbg_h2[w * 128:w * 128 + rows, :], h2e[:rows, :])

            # ---------- AllGather 2 ----------
            nc.gpsimd.collective_compute(
                "AllGather", ALU.bypass, replica_groups=[list(range(NCORES))],
                ins=[ag2_in], outs=[ag2_full])

            # ---------- layer 2 sweep ----------
            for w in range(NW):
                Tw = T[w]
                K = Tw * 128
                off = int(woff[w])
                rows = min(128, NPC - w * 128)

                G2 = sw.tile([128, Tw, EXT2], f32, tag="G")
                nc.gpsimd.dma_gather(G2[:], ag2_full, idx_t[:, off * 8:(off + Tw) * 8],
                                     K, K, EXT2, single_packet=False)

                s01T = build_s01T(Tw, off)
                ps_aDe2 = ps_small.tile([128, Tw * HEADS], f32, space="PSUM", tag="aDe")
                for t in range(Tw):
                    nc.tensor.matmul(ps_aDe2[:, 2 * t:2 * t + 2],
                                     s01T[:, t * 128:(t + 1) * 128],
                                     aD2_res[:, w, :].to_broadcast([128, 2]),
                                     start=True, stop=True)

                sc2 = sw.tile([128, Tw], f32, tag="sc")
                nc.vector.tensor_tensor(
                    sc2[:].rearrange("p (t h) -> p t h", h=1),
                    G2[:, :, OUT_DIM:OUT_DIM + 1],
                    ps_aDe2[:, 0:2 * Tw].rearrange("p (t h) -> p t h", h=2)[:, :, 0:1],
                    op=ALU.add)
                nc.scalar.activation(sc2[:], sc2[:], AF.Prelu, alpha=NEG_SLOPE)
                exp2 = sw.tile([128, Tw], f32r, tag="expw")
                nc.scalar.activation(exp2[:], sc2[:], AF.Exp)

                ps_o2 = ps_out.tile([128, OUT_DIM], f32, space="PSUM", tag="out1")
                ps_den2 = ps_small.tile([128, 2], f32, space="PSUM", tag="den")
                for t in range(Tw):
                    s01 = sg.tile([128, 128], f32r, tag="S01")
                    nc.vector.tensor_tensor(
                        s01[:], io_row[:],
                        dstcol_t[:, off + t:off + t + 1].to_broadcast([128, 128]),
                        op=ALU.is_equal)
                    gp2 = sg.tile([128, OUT_DIM], f32r, tag="Gp")
                    nc.scalar.activation(gp2[:], G2[:, t, 0:OUT_DIM], AF.Copy,
                                         scale=exp2[:, t:t + 1].bitcast(f32))
                    nc.tensor.matmul(ps_o2[:], s01[:], gp2[:],
                                     start=(t == 0), stop=(t == Tw - 1))
                    nc.tensor.matmul(ps_den2[:], s01[:],
                                     exp2[:, t:t + 1].to_broadcast([128, 2]),
                                     start=(t == 0), stop=(t == Tw - 1))

                inv2 = sm.tile([128, 1], f32, tag="inv")
                nc.vector.reciprocal(inv2[:rows, :], ps_den2[:rows, 0:1])
                o2 = sm.tile([128, OUT_DIM], f32, tag="o2")
                nc.vector.tensor_tensor(o2[:rows, :], ps_o2[:rows, :],
                                        inv2[:rows, :].to_broadcast([rows, OUT_DIM]),
                                        op=ALU.mult)
                nc.vector.tensor_tensor(o2[:rows, :], o2[:rows, :], b2_t[:rows, :],
                                        op=ALU.add)
                nc.sync.dma_start(out[w * 128:w * 128 + rows, :], o2[:rows, :])

            sg_ctx.__exit__(None, None, None)
            sm_ctx.__exit__(None, None, None)
            s01T_ctx.__exit__(None, None, None)
            sw_ctx.__exit__(None, None, None)

    nc.compile()
    return nc


def kernel(**inputs):
    global LAST_EXEC_NS
    x = np.ascontiguousarray(np.asarray(inputs["x"], dtype=np.float32))
    edge_index = np.asarray(inputs["edge_index"])
    W1 = np.ascontiguousarray(np.asarray(inputs["W1"], dtype=np.float32))
    a_src1 = np.asarray(inputs["a_src1"], dtype=np.float32)
    a_dst1 = np.asarray(inputs["a_dst1"], dtype=np.float32)
    b1 = np.asarray(inputs["b1"], dtype=np.float32)
    W2 = np.ascontiguousarray(np.asarray(inputs["W2"], dtype=np.float32))
    a_src2 = np.asarray(inputs["a_src2"], dtype=np.float32)
    a_dst2 = np.asarray(inputs["a_dst2"], dtype=np.float32)
    b2 = np.asarray(inputs["b2"], dtype=np.float32)

    debug = bool(int(os.environ.get("GAT_DEBUG", "0")))

    T, tot, idx_np, dstcol_np, dstrow_np = _preprocess(edge_index)
    key = (tuple(T), tot, debug)
    if key not in _CACHE:
        _CACHE[key] = _build(T, tot, debug=debug)
    nc = _CACHE[key]

    def bcast(a, d):
        return np.ascontiguousarray(np.broadcast_to(a.reshape(1, d), (128, d)))

    import ml_dtypes
    common = {
        "W1": W1, "W2": W2,
        "asrc1b": bcast(a_src1, D1), "adst1b": bcast(a_dst1, D1),
        "asrc2b": bcast(a_src2, OUT_DIM), "adst2b": bcast(a_dst2, OUT_DIM),
        "b1b": bcast(b1, D1), "b2b": bcast(b2, OUT_DIM),
        "iota_row": np.ascontiguousarray(
            np.broadcast_to(np.arange(128, dtype=np.float32)[None, :], (128, 128))),
        "iota_col": np.arange(128, dtype=np.float32)[:, None].copy(),
        "ones_row": np.ones((1, 128), ml_dtypes.bfloat16),
        "ident": np.eye(128, dtype=np.float32),
    }
    in_maps = []
    for c in range(NCORES):
        m = dict(common)
        m["xT"] = np.ascontiguousarray(x[c * NPC:(c + 1) * NPC, :].T)
        m["idx"] = idx_np[c]
        m["dstcol"] = np.ascontiguousarray(dstcol_np[c])
        m["dstrow"] = np.ascontiguousarray(dstrow_np[c]).astype(ml_dtypes.bfloat16)
        in_maps.append(m)

    res = run_bass_kernel_spmd(nc, in_maps, core_ids=list(range(NCORES)),
                               trace_cores=[0])
    LAST_EXEC_NS = res.exec_time_ns
    kernel.last_results = res

    return np.concatenate([res.results[c]["out"] for c in range(NCORES)], axis=0)


# revision 15
# speedup vs baseline: 1.2618x; 1.2618x over previous
"""GAT 2-layer kernel for 8 Trainium2 NeuronCores.

Sharding: destination nodes split into 8 contiguous chunks of 2500; each core
owns the segment softmax + weighted scatter for its chunk. Extended feature
tables (h1ext = [h1(512)|aS1(4)|pad], h2ext = [h2(64)|aS2(1)|pad]) are
exchanged via HBM AllGather; per-edge source rows are fetched with dma_gather
(one descriptor per edge); segment softmax + scatter-add are selection
matmuls on the PE (lhsT = 0/1 dst-selection matrix built from iota compares).
float32r (TF32-like) is used on the PE for 1 cycle/row; selection matrices
are exact 0/1 so only the feature operand sees the ~1e-4 rounding.
Softmax skips the max-subtraction (score range ~[-1, 4.1]; exp is safe and
mathematically identical).
"""
import os
import sys
import types

sys.path.insert(0, "/opt/trn_rl_repo")
sys.path.insert(0, "/root/.axon_site")

import numpy as np

try:
    from trn_agent_boot.trn_boot import _ntff_profile_via_ctypes
    _hook = _ntff_profile_via_ctypes('/opt/axon/libaxon_pjrt.so')
    _m = types.ModuleType('antenv.axon_hooks')
    _m.get_axon_ntff_profile_hook = lambda: _hook
    sys.modules['antenv.axon_hooks'] = _m
except Exception:
    pass

import concourse.bacc as bacc
import concourse.mybir as mybir
from concourse.bass_utils import run_bass_kernel_spmd
from concourse.tile import TileContext
from concourse import library_config

f32 = mybir.dt.float32
f32r = mybir.dt.float32r
bf16 = mybir.dt.bfloat16
i16 = mybir.dt.int16
AF = mybir.ActivationFunctionType
ALU = mybir.AluOpType
AX = mybir.AxisListType

N = 20000
IN_DIM = 512
HID = 128
HEADS = 4
D1 = HEADS * HID          # 512
OUT_DIM = 64
NEG_SLOPE = 0.2
NCORES = 8
NPC = N // NCORES         # 2500
NW = (NPC + 127) // 128   # 20
EXT1 = 576                # 2304B rows
EXT2 = 128                # 512B rows

LAST_EXEC_NS = None
_CACHE = {}


def _wrap_idx16(flat):
    K = len(flat)
    out = np.zeros((16, K // 16), np.int16)
    out[np.arange(K) % 16, np.arange(K) // 16] = flat.astype(np.int16)
    return np.tile(out, (8, 1))


SB = 1280          # src-chunk boundary (rows within a core): chunk0=[0,1280) chunk1=[1280,2500)
SZ0, SZ1 = SB, NPC - SB


def _preprocess(edge_index):
    src = np.concatenate([np.asarray(edge_index[0]), np.arange(N)]).astype(np.int64)
    dst = np.concatenate([np.asarray(edge_index[1]), np.arange(N)]).astype(np.int64)

    core = dst // NPC
    dloc = dst % NPC
    win = dloc // 128
    dwin = dloc % 128
    sr = src % NPC
    chunk = (sr >= SB).astype(np.int64)
    # chunk-local table row id
    loc = np.where(chunk == 0, (src // NPC) * SZ0 + sr,
                   (src // NPC) * SZ1 + (sr - SB))

    order = np.lexsort((loc, chunk, win, core))
    cs, ws, js = core[order], win[order], chunk[order]
    srt_loc, srt_dwin = loc[order], dwin[order]
    key = (cs * NW + ws) * 2 + js
    uniq, starts = np.unique(key, return_index=True)
    ends = np.r_[starts[1:], len(key)]
    buckets = {}
    cnt = np.zeros((NCORES, NW, 2), np.int64)
    for u, s, e in zip(uniq, starts, ends):
        cw, j = divmod(int(u), 2)
        c, w = divmod(cw, NW)
        buckets[(c, w, j)] = (srt_loc[s:e], srt_dwin[s:e])
        cnt[c, w, j] = e - s

    T0 = [max(1, int(np.ceil(cnt[:, w, 0].max() / 128.0))) for w in range(NW)]
    T1 = [max(1, int(np.ceil(cnt[:, w, 1].max() / 128.0))) for w in range(NW)]
    tot = sum(T0) + sum(T1)

    idx_np = np.zeros((NCORES, 128, tot * 8), np.int16)
    dstcol_np = np.full((NCORES, 128, tot), -1.0, np.float32)
    dstrow_np = np.full((NCORES, 1, tot * 128), -1.0, np.float32)

    for c in range(NCORES):
        off = 0
        for w in range(NW):
            for j, Tj in ((0, T0[w]), (1, T1[w])):
                K = Tj * 128
                s, d = buckets.get((c, w, j),
                                   (np.zeros(0, np.int64), np.zeros(0, np.int64)))
                spad = np.zeros(K, np.int64)
                spad[:len(s)] = s
                dpad = np.full(K, -1.0, np.float32)
                dpad[:len(d)] = d.astype(np.float32)
                idx_np[c, :, off * 8:(off + Tj) * 8] = _wrap_idx16(spad)
                dstcol_np[c, :, off:off + Tj] = dpad.reshape(Tj, 128).T
                dstrow_np[c, 0, off * 128:(off + Tj) * 128] = dpad
                off += Tj
    return T0, T1, tot, idx_np, dstcol_np, dstrow_np


def _build(T0, T1, tot, debug=False):
    nc = bacc.Bacc("TRN2", target_bir_lowering=False, debug=False,
                   num_devices=NCORES)

    def din(name, shape, dt=f32):
        return nc.dram_tensor(name, shape, dt, kind="ExternalInput").ap()

    xT = din("xT", [IN_DIM, NPC], f32r)
    W1 = din("W1", [IN_DIM, D1], f32r)
    W2 = din("W2", [D1, OUT_DIM], f32r)
    asrc1b = din("asrc1b", [128, D1])
    adst1b = din("adst1b", [128, D1])
    asrc2b = din("asrc2b", [128, OUT_DIM])
    adst2b = din("adst2b", [128, OUT_DIM])
    b1b = din("b1b", [128, D1])
    b2b = din("b2b", [128, OUT_DIM])
    iota_row = din("iota_row", [128, 128])
    iota_col = din("iota_col", [128, 1])
    ones_row = din("ones_row", [1, 128], bf16)
    ident = din("ident", [128, 128])
    idx = din("idx", [128, tot * 8], i16)
    dstcol = din("dstcol", [128, tot])
    dstrow = din("dstrow", [1, tot * 128], bf16)

    out = nc.dram_tensor("out", [NPC, OUT_DIM], f32, kind="ExternalOutput").ap()
    if debug:
        dbg_h1 = nc.dram_tensor("dbg_h1", [NPC, EXT1], f32, kind="ExternalOutput").ap()
        dbg_hr = nc.dram_tensor("dbg_hr", [NPC, D1], f32, kind="ExternalOutput").ap()
        dbg_h2 = nc.dram_tensor("dbg_h2", [NPC, EXT2], f32, kind="ExternalOutput").ap()

    ag1_in0 = nc.dram_tensor("ag1_in0", [SZ0, EXT1], f32).ap()
    ag1_in1 = nc.dram_tensor("ag1_in1", [SZ1, EXT1], f32).ap()
    ag1_c0 = nc.dram_tensor("ag1_c0", [NCORES * SZ0, EXT1], f32, addr_space="Shared").ap()
    ag1_c1 = nc.dram_tensor("ag1_c1", [NCORES * SZ1, EXT1], f32, addr_space="Shared").ap()
    ag2_in0 = nc.dram_tensor("ag2_in0", [SZ0, EXT2], f32).ap()
    ag2_in1 = nc.dram_tensor("ag2_in1", [SZ1, EXT2], f32).ap()
    ag2_c0 = nc.dram_tensor("ag2_c0", [NCORES * SZ0, EXT2], f32, addr_space="Shared").ap()
    ag2_c1 = nc.dram_tensor("ag2_c1", [NCORES * SZ1, EXT2], f32, addr_space="Shared").ap()

    nc.gpsimd.load_library(library_config.mlp)

    Tsum = [a + b for a, b in zip(T0, T1)]
    woff = np.concatenate([[0], np.cumsum(Tsum)]).astype(int)
    NW0 = SZ0 // 128   # windows whose rows go to chunk-0 input (0..9)

    with TileContext(nc) as tc:
        with (
            nc.allow_low_precision(reason="float32r PE operands (TF32-like)"),
            tc.tile_pool(name="consts", bufs=1) as cp,
            tc.tile_pool(name="resident", bufs=1) as rp,
            tc.tile_pool(name="ps_out", bufs=2, space="PSUM") as ps_out,
            tc.tile_pool(name="ps_small", bufs=1, space="PSUM") as ps_small,
            tc.tile_pool(name="ps_bc", bufs=2, space="PSUM") as ps_bc,
        ):
            # ---------- constants ----------
            w2_t = cp.tile([128, 4, OUT_DIM], f32r)
            nc.sync.dma_start(w2_t[:], W2.rearrange("(c k) d -> k c d", k=128))
            asrc2_t = cp.tile([128, OUT_DIM], f32)
            nc.sync.dma_start(asrc2_t[:], asrc2b)
            adst2_t = cp.tile([128, OUT_DIM], f32)
            nc.sync.dma_start(adst2_t[:], adst2b)
            b1_t = cp.tile([128, D1], f32)
            nc.sync.dma_start(b1_t[:], b1b)
            b2_t = cp.tile([128, OUT_DIM], f32)
            nc.sync.dma_start(b2_t[:], b2b)
            io_row = cp.tile([128, 128], f32)
            nc.sync.dma_start(io_row[:], iota_row)
            io_col = cp.tile([128, 1], f32)
            nc.sync.dma_start(io_col[:], iota_col)
            ones_t = cp.tile([1, 128], bf16)
            nc.sync.dma_start(ones_t[:], ones_row)
            id_t = cp.tile([128, 128], f32)
            nc.sync.dma_start(id_t[:], ident)
            idx_t = cp.tile([128, tot * 8], i16)
            nc.sync.dma_start(idx_t[:], idx)
            dstcol_t = cp.tile([128, tot], f32)
            nc.sync.dma_start(dstcol_t[:], dstcol)


            aD1_res = rp.tile([128, NW, HEADS], f32r)
            aD2_res = rp.tile([128, NW, 1], f32r)

            # ---------- phase A ----------
            pA_ctx = tc.tile_pool(name="pA", bufs=3)
            pAc_ctx = tc.tile_pool(name="pAc", bufs=1)
            pA = pA_ctx.__enter__()
            pAc = pAc_ctx.__enter__()
            xt_sb = pAc.tile([128, 4, NPC], f32r)
            nc.sync.dma_start(xt_sb[:], xT.rearrange("(c k) n -> k c n", k=128))
            w1_t = pAc.tile([128, 4, D1], f32r)
            nc.sync.dma_start(w1_t[:], W1.rearrange("(c k) d -> k c d", k=128))
            asrc1_t = pAc.tile([128, D1], f32)
            nc.sync.dma_start(asrc1_t[:], asrc1b)
            adst1_t = pAc.tile([128, D1], f32)
            nc.sync.dma_start(adst1_t[:], adst1b)
            for w in range(NW):
                rows = min(128, NPC - w * 128)
                ps_h1 = ps_out.tile([128, D1], f32, space="PSUM", tag="out1")
                for k in range(4):
                    nc.tensor.matmul(ps_h1[:rows, :],
                                     xt_sb[:, k, w * 128:w * 128 + rows],
                                     w1_t[:, k, :], start=(k == 0), stop=(k == 3))
                h1e = pA.tile([128, EXT1], f32, tag="h1e")
                nc.vector.tensor_copy(h1e[:rows, 0:D1], ps_h1[:rows, :])
                tmp = pA.tile([128, D1], f32, tag="tmpA")
                nc.vector.tensor_tensor(tmp[:rows, :], ps_h1[:rows, :],
                                        asrc1_t[:rows, :], op=ALU.mult)
                for h in range(HEADS):
                    nc.vector.reduce_sum(h1e[:rows, D1 + h:D1 + h + 1],
                                         tmp[:rows, h * HID:(h + 1) * HID], axis=AX.X)
                nc.vector.tensor_tensor(tmp[:rows, :], ps_h1[:rows, :],
                                        adst1_t[:rows, :], op=ALU.mult)
                for h in range(HEADS):
                    nc.vector.reduce_sum(aD1_res[:rows, w, h:h + 1],
                                         tmp[:rows, h * HID:(h + 1) * HID], axis=AX.X)
                if w < NW0:
                    nc.sync.dma_start(ag1_in0[w * 128:w * 128 + rows, :], h1e[:rows, :])
                else:
                    r0 = w * 128 - SB
                    nc.sync.dma_start(ag1_in1[r0:r0 + rows, :], h1e[:rows, :])

            pAc_ctx.__exit__(None, None, None)
            pA_ctx.__exit__(None, None, None)
            sw_ctx = tc.tile_pool(name="sweep", bufs=2)
            s01T_ctx = tc.tile_pool(name="s01Tp", bufs=1)
            sm_ctx = tc.tile_pool(name="small", bufs=2)
            sg_ctx = tc.tile_pool(name="selgp", bufs=6)
            sw = sw_ctx.__enter__()
            s01Tp = s01T_ctx.__enter__()
            sm = sm_ctx.__enter__()
            sg = sg_ctx.__enter__()

            # ---------- AllGather 1 (chunked) ----------
            nc.gpsimd.collective_compute(
                "AllGather", ALU.bypass, replica_groups=[list(range(NCORES))],
                ins=[ag1_in0], outs=[ag1_c0])
            nc.gpsimd.collective_compute(
                "AllGather", ALU.bypass, replica_groups=[list(range(NCORES))],
                ins=[ag1_in1], outs=[ag1_c1])

            if debug:
                for w in range(NW):
                    rows = min(128, NPC - w * 128)
                    t_ = sm.tile([128, EXT1], f32, tag="dbg1")
                    if w < NW0:
                        nc.sync.dma_start(t_[:rows, :], ag1_in0[w * 128:w * 128 + rows, :])
                    else:
                        nc.sync.dma_start(t_[:rows, :], ag1_in1[w * 128 - SB:w * 128 - SB + rows, :])
                    nc.sync.dma_start(dbg_h1[w * 128:w * 128 + rows, :], t_[:rows, :])

            def build_s01T(Tw, off):
                """dst broadcast via K=1 bf16 matmul in 512-wide chunks,
                then compare against the per-partition row index."""
                Kw = Tw * 128
                dr = sw.tile([1, Tw * 128], bf16, tag="dstrow")
                nc.sync.dma_start(dr[:], dstrow[:, off * 128:off * 128 + Kw])
                s01T = s01Tp.tile([128, Tw * 128], f32r, tag="S01T")
                pos = 0
                while pos < Kw:
                    n = min(512, Kw - pos)
                    ps_b = ps_bc.tile([128, 512], f32, space="PSUM", tag="bc")
                    nc.tensor.matmul(ps_b[:, :n], ones_t[:],
                                     dr[:, pos:pos + n],
                                     start=True, stop=True)
                    nc.vector.tensor_tensor(s01T[:, pos:pos + n], ps_b[:, :n],
                                            io_col[:].to_broadcast([128, n]),
                                            op=ALU.is_equal)
                    pos += n
                return s01T

            # ---------- layer 1 sweep + h2 per window ----------
            for w in range(NW):
                Tw = T0[w] + T1[w]
                off = int(woff[w])
                rows = min(128, NPC - w * 128)

                G = sw.tile([128, Tw, EXT1], f32, tag="G")
                nc.gpsimd.dma_gather(G[:, 0:T0[w], :], ag1_c0,
                                     idx_t[:, off * 8:(off + T0[w]) * 8],
                                     T0[w] * 128, T0[w] * 128, EXT1,
                                     single_packet=False)
                nc.gpsimd.dma_gather(G[:, T0[w]:Tw, :], ag1_c1,
                                     idx_t[:, (off + T0[w]) * 8:(off + Tw) * 8],
                                     T1[w] * 128, T1[w] * 128, EXT1,
                                     single_packet=False)

                s01T = build_s01T(Tw, off)
                ps_aDe = ps_small.tile([128, Tw * HEADS], f32, space="PSUM", tag="aDe")
                for t in range(Tw):
                    nc.tensor.matmul(ps_aDe[:, t * HEADS:(t + 1) * HEADS],
                                     s01T[:, t * 128:(t + 1) * 128],
                                     aD1_res[:, w, :], start=True, stop=True)

                # scores -> exp
                sc = sw.tile([128, Tw * HEADS], f32, tag="sc")
                nc.vector.tensor_tensor(
                    sc[:].rearrange("p (t h) -> p t h", h=HEADS),
                    G[:, :, D1:D1 + HEADS],
                    ps_aDe[:].rearrange("p (t h) -> p t h", h=HEADS),
                    op=ALU.add)
                nc.scalar.activation(sc[:], sc[:], AF.Prelu, alpha=NEG_SLOPE)
                expw = sw.tile([128, Tw * HEADS], f32r, tag="expw")
                nc.scalar.activation(expw[:], sc[:], AF.Exp)

                # weighted scatter via selection matmuls
                ps_o1 = ps_out.tile([128, D1], f32, space="PSUM", tag="out1")
                ps_den = ps_small.tile([128, HEADS], f32, space="PSUM", tag="den")
                for t in range(Tw):
                    s01 = sg.tile([128, 128], f32r, tag="S01")
                    nc.vector.tensor_tensor(
                        s01[:], io_row[:],
                        dstcol_t[:, off + t:off + t + 1].to_broadcast([128, 128]),
                        op=ALU.is_equal)
                    gp = sg.tile([128, D1], f32r, tag="Gp")
                    expv = expw[:, t * HEADS:(t + 1) * HEADS].bitcast(f32) \
                        .rearrange("p (h o) -> p h o", o=1) \
                        .to_broadcast([128, HEADS, HID])
                    nc.vector.tensor_tensor(
                        gp[:].rearrange("p (h j) -> p h j", j=HID),
                        G[:, t, 0:D1].rearrange("p (h j) -> p h j", j=HID),
                        expv, op=ALU.mult)
                    nc.tensor.matmul(ps_o1[:], s01[:], gp[:],
                                     start=(t == 0), stop=(t == Tw - 1))
                    nc.tensor.matmul(ps_den[:], s01[:],
                                     expw[:, t * HEADS:(t + 1) * HEADS],
                                     start=(t == 0), stop=(t == Tw - 1))

                # normalize + bias + relu
                inv = sm.tile([128, HEADS], f32, tag="inv")
                nc.vector.reciprocal(inv[:rows, :], ps_den[:rows, :])
                hr_t = sw.tile([128, D1], f32, tag="hr")
                hr = hr_t[:]
                nc.vector.tensor_tensor(
                    hr.rearrange("p (h j) -> p h j", j=HID)[:rows],
                    ps_o1[:rows, :].rearrange("p (h j) -> p h j", j=HID),
                    inv[:rows, :].rearrange("p (h o) -> p h o", o=1)
                        .to_broadcast([rows, HEADS, HID]),
                    op=ALU.mult)
                nc.vector.tensor_tensor(hr[:rows], hr[:rows], b1_t[:rows, :],
                                        op=ALU.add)
                nc.scalar.activation(hr[:rows], hr[:rows], AF.Relu)

                # h2 = hrelu @ W2 via PE transpose
                ps_h2 = ps_small.tile([128, OUT_DIM], f32, space="PSUM", tag="h2")
                for k in range(4):
                    ps_tr = ps_bc.tile([128, 512], f32, space="PSUM", tag="bc")
                    nc.tensor.transpose(ps_tr[:, :rows],
                                        hr[:rows, k * 128:(k + 1) * 128],
                                        id_t[:rows, :rows])
                    hT = sm.tile([128, 128], f32r, tag="hT")
                    nc.vector.tensor_copy(hT[:, :rows], ps_tr[:, :rows])
                    nc.tensor.matmul(ps_h2[:rows, :], hT[:, :rows], w2_t[:, k, :],
                                     start=(k == 0), stop=(k == 3))
                h2e = sm.tile([128, EXT2], f32, tag="h2e")
                nc.vector.tensor_copy(h2e[:rows, 0:OUT_DIM], ps_h2[:rows, :])
                tmp2 = sm.tile([128, OUT_DIM], f32, tag="tmp2")
                nc.vector.tensor_tensor(tmp2[:rows, :], ps_h2[:rows, :],
                                        asrc2_t[:rows, :], op=ALU.mult)
                nc.vector.reduce_sum(h2e[:rows, OUT_DIM:OUT_DIM + 1],
                                     tmp2[:rows, :], axis=AX.X)
                nc.vector.tensor_tensor(tmp2[:rows, :], ps_h2[:rows, :],
                                        adst2_t[:rows, :], op=ALU.mult)
                nc.vector.reduce_sum(aD2_res[:rows, w, :], tmp2[:rows, :], axis=AX.X)
                if w < NW0:
                    nc.sync.dma_start(ag2_in0[w * 128:w * 128 + rows, :], h2e[:rows, :])
                else:
                    r0 = w * 128 - SB
                    nc.sync.dma_start(ag2_in1[r0:r0 + rows, :], h2e[:rows, :])

                if debug:
                    nc.sync.dma_start(dbg_hr[w * 128:w * 128 + rows, :], hr[:rows])
                    nc.sync.dma_start(dbg_h2[w * 128:w * 128 + rows, :], h2e[:rows, :])

            # ---------- AllGather 2 (chunked) ----------
            nc.gpsimd.collective_compute(
                "AllGather", ALU.bypass, replica_groups=[list(range(NCORES))],
                ins=[ag2_in0], outs=[ag2_c0])
            nc.gpsimd.collective_compute(
                "AllGather", ALU.bypass, replica_groups=[list(range(NCORES))],
                ins=[ag2_in1], outs=[ag2_c1])

            # ---------- layer 2 sweep ----------
            for w in range(NW):
                Tw = T0[w] + T1[w]
                off = int(woff[w])
                rows = min(128, NPC - w * 128)

                G2 = sw.tile([128, Tw, EXT2], f32, tag="G")
                nc.gpsimd.dma_gather(G2[:, 0:T0[w], :], ag2_c0,
                                     idx_t[:, off * 8:(off + T0[w]) * 8],
                                     T0[w] * 128, T0[w] * 128, EXT2,
                                     single_packet=False)
                nc.gpsimd.dma_gather(G2[:, T0[w]:Tw, :], ag2_c1,
                                     idx_t[:, (off + T0[w]) * 8:(off + Tw) * 8],
                                     T1[w] * 128, T1[w] * 128, EXT2,
                                     single_packet=False)

                s01T = build_s01T(Tw, off)
                ps_aDe2 = ps_small.tile([128, Tw * HEADS], f32, space="PSUM", tag="aDe")
                for t in range(Tw):
                    nc.tensor.matmul(ps_aDe2[:, 2 * t:2 * t + 2],
                                     s01T[:, t * 128:(t + 1) * 128],
                                     aD2_res[:, w, :].to_broadcast([128, 2]),
                                     start=True, stop=True)

                sc2 = sw.tile([128, Tw], f32, tag="sc")
                nc.vector.tensor_tensor(
                    sc2[:].rearrange("p (t h) -> p t h", h=1),
                    G2[:, :, OUT_DIM:OUT_DIM + 1],
                    ps_aDe2[:, 0:2 * Tw].rearrange("p (t h) -> p t h", h=2)[:, :, 0:1],
                    op=ALU.add)
                nc.scalar.activation(sc2[:], sc2[:], AF.Prelu, alpha=NEG_SLOPE)
                exp2 = sw.tile([128, Tw], f32r, tag="expw")
                nc.scalar.activation(exp2[:], sc2[:], AF.Exp)

                ps_o2 = ps_out.tile([128, OUT_DIM], f32, space="PSUM", tag="out1")
                ps_den2 = ps_small.tile([128, 2], f32, space="PSUM", tag="den")
                for t in range(Tw):
                    s01 = sg.tile([128, 128], f32r, tag="S01")
                    nc.vector.tensor_tensor(
                        s01[:], io_row[:],
                        dstcol_t[:, off + t:off + t + 1].to_broadcast([128, 128]),
                        op=ALU.is_equal)
                    gp2 = sg.tile([128, OUT_DIM], f32r, tag="Gp")
                    nc.vector.tensor_tensor(
                        gp2[:], G2[:, t, 0:OUT_DIM],
                        exp2[:, t:t + 1].bitcast(f32).to_broadcast([128, OUT_DIM]),
                        op=ALU.mult)
                    nc.tensor.matmul(ps_o2[:], s01[:], gp2[:],
                                     start=(t == 0), stop=(t == Tw - 1))
                    nc.tensor.matmul(ps_den2[:], s01[:],
                                     exp2[:, t:t + 1].to_broadcast([128, 2]),
                                     start=(t == 0), stop=(t == Tw - 1))

                inv2 = sm.tile([128, 1], f32, tag="inv")
                nc.vector.reciprocal(inv2[:rows, :], ps_den2[:rows, 0:1])
                o2 = sm.tile([128, OUT_DIM], f32, tag="o2")
                nc.vector.tensor_tensor(o2[:rows, :], ps_o2[:rows, :],
                                        inv2[:rows, :].to_broadcast([rows, OUT_DIM]),
                                        op=ALU.mult)
                nc.vector.tensor_tensor(o2[:rows, :], o2[:rows, :], b2_t[:rows, :],
                                        op=ALU.add)
                nc.sync.dma_start(out[w * 128:w * 128 + rows, :], o2[:rows, :])

            sg_ctx.__exit__(None, None, None)
            sm_ctx.__exit__(None, None, None)
            s01T_ctx.__exit__(None, None, None)
            sw_ctx.__exit__(None, None, None)

    nc.compile()
    return nc


def kernel(**inputs):
    global LAST_EXEC_NS
    x = np.ascontiguousarray(np.asarray(inputs["x"], dtype=np.float32))
    edge_index = np.asarray(inputs["edge_index"])
    W1 = np.ascontiguousarray(np.asarray(inputs["W1"], dtype=np.float32))
    a_src1 = np.asarray(inputs["a_src1"], dtype=np.float32)
    a_dst1 = np.asarray(inputs["a_dst1"], dtype=np.float32)
    b1 = np.asarray(inputs["b1"], dtype=np.float32)
    W2 = np.ascontiguousarray(np.asarray(inputs["W2"], dtype=np.float32))
    a_src2 = np.asarray(inputs["a_src2"], dtype=np.float32)
    a_dst2 = np.asarray(inputs["a_dst2"], dtype=np.float32)
    b2 = np.asarray(inputs["b2"], dtype=np.float32)

    debug = bool(int(os.environ.get("GAT_DEBUG", "0")))

    T0, T1, tot, idx_np, dstcol_np, dstrow_np = _preprocess(edge_index)
    key = (tuple(T0), tuple(T1), tot, debug)
    if key not in _CACHE:
        _CACHE[key] = _build(T0, T1, tot, debug=debug)
    nc = _CACHE[key]

    def bcast(a, d):
        return np.ascontiguousarray(np.broadcast_to(a.reshape(1, d), (128, d)))

    import ml_dtypes
    common = {
        "W1": W1, "W2": W2,
        "asrc1b": bcast(a_src1, D1), "adst1b": bcast(a_dst1, D1),
        "asrc2b": bcast(a_src2, OUT_DIM), "adst2b": bcast(a_dst2, OUT_DIM),
        "b1b": bcast(b1, D1), "b2b": bcast(b2, OUT_DIM),
        "iota_row": np.ascontiguousarray(
            np.broadcast_to(np.arange(128, dtype=np.float32)[None, :], (128, 128))),
        "iota_col": np.arange(128, dtype=np.float32)[:, None].copy(),
        "ones_row": np.ones((1, 128), ml_dtypes.bfloat16),
        "ident": np.eye(128, dtype=np.float32),
    }
    in_maps = []
    for c in range(NCORES):
        m = dict(common)
        m["xT"] = np.ascontiguousarray(x[c * NPC:(c + 1) * NPC, :].T)
        m["idx"] = idx_np[c]
        m["dstcol"] = np.ascontiguousarray(dstcol_np[c])
        m["dstrow"] = np.ascontiguousarray(dstrow_np[c]).astype(ml_dtypes.bfloat16)
        in_maps.append(m)

    res = run_bass_kernel_spmd(nc, in_maps, core_ids=list(range(NCORES)),
                               trace_cores=[0])
    LAST_EXEC_NS = res.exec_time_ns
    kernel.last_results = res

    return np.concatenate([res.results[c]["out"] for c in range(NCORES)], axis=0)


# revision 17
# speedup vs baseline: 1.3246x; 1.0498x over previous
"""GAT 2-layer kernel for 8 Trainium2 NeuronCores.

Sharding: destination nodes split into 8 contiguous chunks of 2500; each core
owns the segment softmax + weighted scatter for its chunk. Extended feature
tables (h1ext = [h1(512)|aS1(4)|pad], h2ext = [h2(64)|aS2(1)|pad]) are
exchanged via HBM AllGather; per-edge source rows are fetched with dma_gather
(one descriptor per edge); segment softmax + scatter-add are selection
matmuls on the PE (lhsT = 0/1 dst-selection matrix built from iota compares).
float32r (TF32-like) is used on the PE for 1 cycle/row; selection matrices
are exact 0/1 so only the feature operand sees the ~1e-4 rounding.
Softmax skips the max-subtraction (score range ~[-1, 4.1]; exp is safe and
mathematically identical).
"""
import os
import sys
import types

sys.path.insert(0, "/opt/trn_rl_repo")
sys.path.insert(0, "/root/.axon_site")

import numpy as np

try:
    from trn_agent_boot.trn_boot import _ntff_profile_via_ctypes
    _hook = _ntff_profile_via_ctypes('/opt/axon/libaxon_pjrt.so')
    _m = types.ModuleType('antenv.axon_hooks')
    _m.get_axon_ntff_profile_hook = lambda: _hook
    sys.modules['antenv.axon_hooks'] = _m
except Exception:
    pass

import concourse.bacc as bacc
import concourse.mybir as mybir
from concourse.bass_utils import run_bass_kernel_spmd
from concourse.tile import TileContext
from concourse import library_config

f32 = mybir.dt.float32
f32r = mybir.dt.float32r
bf16 = mybir.dt.bfloat16
i16 = mybir.dt.int16
AF = mybir.ActivationFunctionType
ALU = mybir.AluOpType
AX = mybir.AxisListType

N = 20000
IN_DIM = 512
HID = 128
HEADS = 4
D1 = HEADS * HID          # 512
OUT_DIM = 64
NEG_SLOPE = 0.2
NCORES = 8
NPC = N // NCORES         # 2500
NW = (NPC + 127) // 128   # 20
EXT1 = 576                # 2304B rows
EXT2 = 128                # 512B rows

LAST_EXEC_NS = None
_CACHE = {}


def _wrap_idx16(flat):
    K = len(flat)
    out = np.zeros((16, K // 16), np.int16)
    out[np.arange(K) % 16, np.arange(K) // 16] = flat.astype(np.int16)
    return np.tile(out, (8, 1))


SB = 1280          # src-chunk boundary (rows within a core): chunk0=[0,1280) chunk1=[1280,2500)
SZ0, SZ1 = SB, NPC - SB


def _preprocess(edge_index):
    src = np.concatenate([np.asarray(edge_index[0]), np.arange(N)]).astype(np.int64)
    dst = np.concatenate([np.asarray(edge_index[1]), np.arange(N)]).astype(np.int64)

    core = dst // NPC
    dloc = dst % NPC
    win = dloc // 128
    dwin = dloc % 128
    sr = src % NPC
    chunk = (sr >= SB).astype(np.int64)
    # chunk-local table row id
    loc = np.where(chunk == 0, (src // NPC) * SZ0 + sr,
                   (src // NPC) * SZ1 + (sr - SB))

    order = np.lexsort((loc, chunk, win, core))
    cs, ws, js = core[order], win[order], chunk[order]
    srt_loc, srt_dwin = loc[order], dwin[order]
    key = (cs * NW + ws) * 2 + js
    uniq, starts = np.unique(key, return_index=True)
    ends = np.r_[starts[1:], len(key)]
    buckets = {}
    cnt = np.zeros((NCORES, NW, 2), np.int64)
    for u, s, e in zip(uniq, starts, ends):
        cw, j = divmod(int(u), 2)
        c, w = divmod(cw, NW)
        buckets[(c, w, j)] = (srt_loc[s:e], srt_dwin[s:e])
        cnt[c, w, j] = e - s

    T0 = [max(1, int(np.ceil(cnt[:, w, 0].max() / 128.0))) for w in range(NW)]
    T1 = [max(1, int(np.ceil(cnt[:, w, 1].max() / 128.0))) for w in range(NW)]
    tot = sum(T0) + sum(T1)

    idx_np = np.zeros((NCORES, 128, tot * 8), np.int16)
    dstcol_np = np.full((NCORES, 128, tot), -1.0, np.float32)
    dstrow_np = np.full((NCORES, 1, tot * 128), -1.0, np.float32)

    for c in range(NCORES):
        off = 0
        for w in range(NW):
            for j, Tj in ((0, T0[w]), (1, T1[w])):
                K = Tj * 128
                s, d = buckets.get((c, w, j),
                                   (np.zeros(0, np.int64), np.zeros(0, np.int64)))
                spad = np.zeros(K, np.int64)
                spad[:len(s)] = s
                dpad = np.full(K, -1.0, np.float32)
                dpad[:len(d)] = d.astype(np.float32)
                idx_np[c, :, off * 8:(off + Tj) * 8] = _wrap_idx16(spad)
                dstcol_np[c, :, off:off + Tj] = dpad.reshape(Tj, 128).T
                dstrow_np[c, 0, off * 128:(off + Tj) * 128] = dpad
                off += Tj
    return T0, T1, tot, idx_np, dstcol_np, dstrow_np


def _build(T0, T1, tot, debug=False):
    nc = bacc.Bacc("TRN2", target_bir_lowering=False, debug=False,
                   num_devices=NCORES)

    def din(name, shape, dt=f32):
        return nc.dram_tensor(name, shape, dt, kind="ExternalInput").ap()

    xT = din("xT", [IN_DIM, NPC], f32r)
    W1 = din("W1", [IN_DIM, D1], f32r)
    W2 = din("W2", [D1, OUT_DIM], f32r)
    asrc1b = din("asrc1b", [128, D1])
    adst1b = din("adst1b", [128, D1])
    asrc2b = din("asrc2b", [128, OUT_DIM])
    adst2b = din("adst2b", [128, OUT_DIM])
    b1b = din("b1b", [128, D1])
    b2b = din("b2b", [128, OUT_DIM])
    iota_row = din("iota_row", [128, 128])
    iota_col = din("iota_col", [128, 1])
    ones_row = din("ones_row", [1, 128], bf16)
    ident = din("ident", [128, 128])
    idx = din("idx", [128, tot * 8], i16)
    dstcol = din("dstcol", [128, tot])
    dstrow = din("dstrow", [1, tot * 128], bf16)

    out = nc.dram_tensor("out", [NPC, OUT_DIM], f32, kind="ExternalOutput").ap()
    if debug:
        dbg_h1 = nc.dram_tensor("dbg_h1", [NPC, EXT1], f32, kind="ExternalOutput").ap()
        dbg_hr = nc.dram_tensor("dbg_hr", [NPC, D1], f32, kind="ExternalOutput").ap()
        dbg_h2 = nc.dram_tensor("dbg_h2", [NPC, EXT2], f32, kind="ExternalOutput").ap()

    ag1_in0 = nc.dram_tensor("ag1_in0", [SZ0, EXT1], f32).ap()
    ag1_in1 = nc.dram_tensor("ag1_in1", [SZ1, EXT1], f32).ap()
    ag1_c0 = nc.dram_tensor("ag1_c0", [NCORES * SZ0, EXT1], f32, addr_space="Shared").ap()
    ag1_c1 = nc.dram_tensor("ag1_c1", [NCORES * SZ1, EXT1], f32, addr_space="Shared").ap()
    ag2_in0 = nc.dram_tensor("ag2_in0", [SZ0, EXT2], f32).ap()
    ag2_in1 = nc.dram_tensor("ag2_in1", [SZ1, EXT2], f32).ap()
    ag2_c0 = nc.dram_tensor("ag2_c0", [NCORES * SZ0, EXT2], f32, addr_space="Shared").ap()
    ag2_c1 = nc.dram_tensor("ag2_c1", [NCORES * SZ1, EXT2], f32, addr_space="Shared").ap()

    nc.gpsimd.load_library(library_config.mlp)

    Tsum = [a + b for a, b in zip(T0, T1)]
    woff = np.concatenate([[0], np.cumsum(Tsum)]).astype(int)
    NW0 = SZ0 // 128   # windows whose rows go to chunk-0 input (0..9)

    with TileContext(nc) as tc:
        with (
            nc.allow_low_precision(reason="float32r PE operands (TF32-like)"),
            tc.tile_pool(name="consts", bufs=1) as cp,
            tc.tile_pool(name="resident", bufs=1) as rp,
            tc.tile_pool(name="ps_out", bufs=2, space="PSUM") as ps_out,
            tc.tile_pool(name="ps_small", bufs=1, space="PSUM") as ps_small,
            tc.tile_pool(name="ps_bc", bufs=2, space="PSUM") as ps_bc,
        ):
            # ---------- constants ----------
            w2_t = cp.tile([128, 4, OUT_DIM], f32r)
            nc.sync.dma_start(w2_t[:], W2.rearrange("(c k) d -> k c d", k=128))
            asrc2_t = cp.tile([128, OUT_DIM], f32)
            nc.sync.dma_start(asrc2_t[:], asrc2b)
            adst2_t = cp.tile([128, OUT_DIM], f32)
            nc.sync.dma_start(adst2_t[:], adst2b)
            b1_t = cp.tile([128, D1], f32)
            nc.sync.dma_start(b1_t[:], b1b)
            b2_t = cp.tile([128, OUT_DIM], f32)
            nc.sync.dma_start(b2_t[:], b2b)
            io_row = cp.tile([128, 128], f32)
            nc.sync.dma_start(io_row[:], iota_row)
            io_col = cp.tile([128, 1], f32)
            nc.sync.dma_start(io_col[:], iota_col)
            ones_t = cp.tile([1, 128], bf16)
            nc.sync.dma_start(ones_t[:], ones_row)
            id_t = cp.tile([128, 128], f32)
            nc.sync.dma_start(id_t[:], ident)
            idx_t = cp.tile([128, tot * 8], i16)
            nc.sync.dma_start(idx_t[:], idx)
            dstcol_t = cp.tile([128, tot], f32)
            nc.sync.dma_start(dstcol_t[:], dstcol)


            aD1_res = rp.tile([128, NW, HEADS], f32r)
            aD2_res = rp.tile([128, NW, 1], f32r)

            # ---------- phase A ----------
            pA_ctx = tc.tile_pool(name="pA", bufs=3)
            pAc_ctx = tc.tile_pool(name="pAc", bufs=1)
            pA = pA_ctx.__enter__()
            pAc = pAc_ctx.__enter__()
            xt_sb = pAc.tile([128, 4, NPC], f32r)
            nc.sync.dma_start(xt_sb[:], xT.rearrange("(c k) n -> k c n", k=128))
            w1_t = pAc.tile([128, 4, D1], f32r)
            nc.sync.dma_start(w1_t[:], W1.rearrange("(c k) d -> k c d", k=128))
            asrc1_t = pAc.tile([128, D1], f32)
            nc.sync.dma_start(asrc1_t[:], asrc1b)
            adst1_t = pAc.tile([128, D1], f32)
            nc.sync.dma_start(adst1_t[:], adst1b)
            for w in range(NW):
                rows = min(128, NPC - w * 128)
                ps_h1 = ps_out.tile([128, D1], f32, space="PSUM", tag="out1")
                for k in range(4):
                    nc.tensor.matmul(ps_h1[:rows, :],
                                     xt_sb[:, k, w * 128:w * 128 + rows],
                                     w1_t[:, k, :], start=(k == 0), stop=(k == 3))
                h1e = pA.tile([128, EXT1], f32, tag="h1e")
                nc.vector.tensor_copy(h1e[:rows, 0:D1], ps_h1[:rows, :])
                tmp = pA.tile([128, D1], f32, tag="tmpA")
                nc.vector.tensor_tensor(tmp[:rows, :], ps_h1[:rows, :],
                                        asrc1_t[:rows, :], op=ALU.mult)
                for h in range(HEADS):
                    nc.vector.reduce_sum(h1e[:rows, D1 + h:D1 + h + 1],
                                         tmp[:rows, h * HID:(h + 1) * HID], axis=AX.X)
                nc.vector.tensor_tensor(tmp[:rows, :], ps_h1[:rows, :],
                                        adst1_t[:rows, :], op=ALU.mult)
                for h in range(HEADS):
                    nc.vector.reduce_sum(aD1_res[:rows, w, h:h + 1],
                                         tmp[:rows, h * HID:(h + 1) * HID], axis=AX.X)
                if w < NW0:
                    nc.sync.dma_start(ag1_in0[w * 128:w * 128 + rows, :], h1e[:rows, :])
                else:
                    r0 = w * 128 - SB
                    nc.sync.dma_start(ag1_in1[r0:r0 + rows, :], h1e[:rows, :])

            pAc_ctx.__exit__(None, None, None)
            pA_ctx.__exit__(None, None, None)
            sw_ctx = tc.tile_pool(name="sweep", bufs=2)
            s01T_ctx = tc.tile_pool(name="s01Tp", bufs=1)
            sm_ctx = tc.tile_pool(name="small", bufs=2)
            sg_ctx = tc.tile_pool(name="selgp", bufs=6)
            sw = sw_ctx.__enter__()
            s01Tp = s01T_ctx.__enter__()
            sm = sm_ctx.__enter__()
            sg = sg_ctx.__enter__()

            # ---------- AllGather 1 (chunked) ----------
            nc.gpsimd.collective_compute(
                "AllGather", ALU.bypass, replica_groups=[list(range(NCORES))],
                ins=[ag1_in0], outs=[ag1_c0])
            nc.gpsimd.collective_compute(
                "AllGather", ALU.bypass, replica_groups=[list(range(NCORES))],
                ins=[ag1_in1], outs=[ag1_c1])

            if debug:
                for w in range(NW):
                    rows = min(128, NPC - w * 128)
                    t_ = sm.tile([128, EXT1], f32, tag="dbg1")
                    if w < NW0:
                        nc.sync.dma_start(t_[:rows, :], ag1_in0[w * 128:w * 128 + rows, :])
                    else:
                        nc.sync.dma_start(t_[:rows, :], ag1_in1[w * 128 - SB:w * 128 - SB + rows, :])
                    nc.sync.dma_start(dbg_h1[w * 128:w * 128 + rows, :], t_[:rows, :])

            def build_s01T(Tw, off):
                """dst broadcast via K=1 bf16 matmul in 512-wide chunks,
                then compare against the per-partition row index."""
                Kw = Tw * 128
                dr = sw.tile([1, Tw * 128], bf16, tag="dstrow")
                nc.sync.dma_start(dr[:], dstrow[:, off * 128:off * 128 + Kw])
                s01T = s01Tp.tile([128, Tw * 128], f32r, tag="S01T")
                pos = 0
                while pos < Kw:
                    n = min(512, Kw - pos)
                    ps_b = ps_bc.tile([128, 512], f32, space="PSUM", tag="bc")
                    nc.tensor.matmul(ps_b[:, :n], ones_t[:],
                                     dr[:, pos:pos + n],
                                     start=True, stop=True)
                    nc.vector.tensor_tensor(s01T[:, pos:pos + n], ps_b[:, :n],
                                            io_col[:].to_broadcast([128, n]),
                                            op=ALU.is_equal)
                    pos += n
                return s01T

            # ---------- layer 1 sweep + h2 per window ----------
            for w in range(NW):
                off = int(woff[w])
                rows = min(128, NPC - w * 128)

                ps_o1 = ps_out.tile([128, D1], f32, space="PSUM", tag="out1")
                ps_den = ps_small.tile([128, HEADS], f32, space="PSUM", tag="den")
                for j, Tj, tab in ((0, T0[w], ag1_c0), (1, T1[w], ag1_c1)):
                    oj = off if j == 0 else off + T0[w]
                    G = sw.tile([128, Tj, EXT1], f32, tag="G" + str(j))
                    nc.gpsimd.dma_gather(G[:], tab,
                                         idx_t[:, oj * 8:(oj + Tj) * 8],
                                         Tj * 128, Tj * 128, EXT1,
                                         single_packet=False)

                    s01T = build_s01T(Tj, oj)
                    ps_aDe = ps_small.tile([128, Tj * HEADS], f32, space="PSUM",
                                           tag="aDe")
                    for t in range(Tj):
                        nc.tensor.matmul(ps_aDe[:, t * HEADS:(t + 1) * HEADS],
                                         s01T[:, t * 128:(t + 1) * 128],
                                         aD1_res[:, w, :], start=True, stop=True)

                    sc = sw.tile([128, Tj * HEADS], f32, tag="sc")
                    nc.vector.tensor_tensor(
                        sc[:].rearrange("p (t h) -> p t h", h=HEADS),
                        G[:, :, D1:D1 + HEADS],
                        ps_aDe[:].rearrange("p (t h) -> p t h", h=HEADS),
                        op=ALU.add)
                    nc.scalar.activation(sc[:], sc[:], AF.Prelu, alpha=NEG_SLOPE)
                    expw = sw.tile([128, Tj * HEADS], f32r, tag="expw")
                    nc.scalar.activation(expw[:], sc[:], AF.Exp)

                    for t in range(Tj):
                        s01 = sg.tile([128, 128], f32r, tag="S01")
                        nc.vector.tensor_tensor(
                            s01[:], io_row[:],
                            dstcol_t[:, oj + t:oj + t + 1].to_broadcast([128, 128]),
                            op=ALU.is_equal)
                        gp = sg.tile([128, D1], f32r, tag="Gp")
                        expv = expw[:, t * HEADS:(t + 1) * HEADS].bitcast(f32) \
                            .rearrange("p (h o) -> p h o", o=1) \
                            .to_broadcast([128, HEADS, HID])
                        nc.vector.tensor_tensor(
                            gp[:].rearrange("p (h j) -> p h j", j=HID),
                            G[:, t, 0:D1].rearrange("p (h j) -> p h j", j=HID),
                            expv, op=ALU.mult)
                        st = (j == 0 and t == 0)
                        sp = (j == 1 and t == Tj - 1)
                        nc.tensor.matmul(ps_o1[:], s01[:], gp[:],
                                         start=st, stop=sp)
                        nc.tensor.matmul(ps_den[:], s01[:],
                                         expw[:, t * HEADS:(t + 1) * HEADS],
                                         start=st, stop=sp)

                # normalize + bias + relu
                inv = sm.tile([128, HEADS], f32, tag="inv")
                nc.vector.reciprocal(inv[:rows, :], ps_den[:rows, :])
                hr_t = sw.tile([128, D1], f32, tag="hr")
                hr = hr_t[:]
                nc.vector.tensor_tensor(
                    hr.rearrange("p (h j) -> p h j", j=HID)[:rows],
                    ps_o1[:rows, :].rearrange("p (h j) -> p h j", j=HID),
                    inv[:rows, :].rearrange("p (h o) -> p h o", o=1)
                        .to_broadcast([rows, HEADS, HID]),
                    op=ALU.mult)
                nc.vector.tensor_tensor(hr[:rows], hr[:rows], b1_t[:rows, :],
                                        op=ALU.add)
                nc.scalar.activation(hr[:rows], hr[:rows], AF.Relu)

                # h2 = hrelu @ W2 via PE transpose
                ps_h2 = ps_small.tile([128, OUT_DIM], f32, space="PSUM", tag="h2")
                for k in range(4):
                    ps_tr = ps_bc.tile([128, 512], f32, space="PSUM", tag="bc")
                    nc.tensor.transpose(ps_tr[:, :rows],
                                        hr[:rows, k * 128:(k + 1) * 128],
                                        id_t[:rows, :rows])
                    hT = sm.tile([128, 128], f32r, tag="hT")
                    nc.vector.tensor_copy(hT[:, :rows], ps_tr[:, :rows])
                    nc.tensor.matmul(ps_h2[:rows, :], hT[:, :rows], w2_t[:, k, :],
                                     start=(k == 0), stop=(k == 3))
                h2e = sm.tile([128, EXT2], f32, tag="h2e")
                nc.vector.tensor_copy(h2e[:rows, 0:OUT_DIM], ps_h2[:rows, :])
                tmp2 = sm.tile([128, OUT_DIM], f32, tag="tmp2")
                nc.vector.tensor_tensor(tmp2[:rows, :], ps_h2[:rows, :],
                                        asrc2_t[:rows, :], op=ALU.mult)
                nc.vector.reduce_sum(h2e[:rows, OUT_DIM:OUT_DIM + 1],
                                     tmp2[:rows, :], axis=AX.X)
                nc.vector.tensor_tensor(tmp2[:rows, :], ps_h2[:rows, :],
                                        adst2_t[:rows, :], op=ALU.mult)
                nc.vector.reduce_sum(aD2_res[:rows, w, :], tmp2[:rows, :], axis=AX.X)
                if w < NW0:
                    nc.sync.dma_start(ag2_in0[w * 128:w * 128 + rows, :], h2e[:rows, :])
                else:
                    r0 = w * 128 - SB
                    nc.sync.dma_start(ag2_in1[r0:r0 + rows, :], h2e[:rows, :])

                if debug:
                    nc.sync.dma_start(dbg_hr[w * 128:w * 128 + rows, :], hr[:rows])
                    nc.sync.dma_start(dbg_h2[w * 128:w * 128 + rows, :], h2e[:rows, :])

            # ---------- AllGather 2 (chunked) ----------
            nc.gpsimd.collective_compute(
                "AllGather", ALU.bypass, replica_groups=[list(range(NCORES))],
                ins=[ag2_in0], outs=[ag2_c0])
            nc.gpsimd.collective_compute(
                "AllGather", ALU.bypass, replica_groups=[list(range(NCORES))],
                ins=[ag2_in1], outs=[ag2_c1])

            # ---------- layer 2 sweep ----------
            for w in range(NW):
                off = int(woff[w])
                rows = min(128, NPC - w * 128)

                ps_o2 = ps_out.tile([128, OUT_DIM], f32, space="PSUM", tag="out1")
                ps_den2 = ps_small.tile([128, 2], f32, space="PSUM", tag="den")
                for j, Tj, tab in ((0, T0[w], ag2_c0), (1, T1[w], ag2_c1)):
                    oj = off if j == 0 else off + T0[w]
                    G2 = sw.tile([128, Tj, EXT2], f32, tag="G" + str(j))
                    nc.gpsimd.dma_gather(G2[:], tab,
                                         idx_t[:, oj * 8:(oj + Tj) * 8],
                                         Tj * 128, Tj * 128, EXT2,
                                         single_packet=False)

                    s01T = build_s01T(Tj, oj)
                    ps_aDe2 = ps_small.tile([128, Tj * HEADS], f32, space="PSUM",
                                            tag="aDe")
                    for t in range(Tj):
                        nc.tensor.matmul(ps_aDe2[:, 2 * t:2 * t + 2],
                                         s01T[:, t * 128:(t + 1) * 128],
                                         aD2_res[:, w, :].to_broadcast([128, 2]),
                                         start=True, stop=True)

                    sc2 = sw.tile([128, Tj], f32, tag="sc")
                    nc.vector.tensor_tensor(
                        sc2[:].rearrange("p (t h) -> p t h", h=1),
                        G2[:, :, OUT_DIM:OUT_DIM + 1],
                        ps_aDe2[:, 0:2 * Tj].rearrange("p (t h) -> p t h", h=2)[:, :, 0:1],
                        op=ALU.add)
                    nc.scalar.activation(sc2[:], sc2[:], AF.Prelu, alpha=NEG_SLOPE)
                    exp2 = sw.tile([128, Tj], f32r, tag="expw")
                    nc.scalar.activation(exp2[:], sc2[:], AF.Exp)

                    for t in range(Tj):
                        s01 = sg.tile([128, 128], f32r, tag="S01")
                        nc.vector.tensor_tensor(
                            s01[:], io_row[:],
                            dstcol_t[:, oj + t:oj + t + 1].to_broadcast([128, 128]),
                            op=ALU.is_equal)
                        gp2 = sg.tile([128, OUT_DIM], f32r, tag="Gp")
                        nc.vector.tensor_tensor(
                            gp2[:], G2[:, t, 0:OUT_DIM],
                            exp2[:, t:t + 1].bitcast(f32).to_broadcast([128, OUT_DIM]),
                            op=ALU.mult)
                        st = (j == 0 and t == 0)
                        sp = (j == 1 and t == Tj - 1)
                        nc.tensor.matmul(ps_o2[:], s01[:], gp2[:],
                                         start=st, stop=sp)
                        nc.tensor.matmul(ps_den2[:], s01[:],
                                         exp2[:, t:t + 1].to_broadcast([128, 2]),
                                         start=st, stop=sp)

                inv2 = sm.tile([128, 1], f32, tag="inv")
                nc.vector.reciprocal(inv2[:rows, :], ps_den2[:rows, 0:1])
                o2 = sm.tile([128, OUT_DIM], f32, tag="o2")
                nc.vector.tensor_tensor(o2[:rows, :], ps_o2[:rows, :],
                                        inv2[:rows, :].to_broadcast([rows, OUT_DIM]),
                                        op=ALU.mult)
                nc.vector.tensor_tensor(o2[:rows, :], o2[:rows, :], b2_t[:rows, :],
                                        op=ALU.add)
                nc.sync.dma_start(out[w * 128:w * 128 + rows, :], o2[:rows, :])

            sg_ctx.__exit__(None, None, None)
            sm_ctx.__exit__(None, None, None)
            s01T_ctx.__exit__(None, None, None)
            sw_ctx.__exit__(None, None, None)

    nc.compile()
    return nc


def kernel(**inputs):
    global LAST_EXEC_NS
    x = np.ascontiguousarray(np.asarray(inputs["x"], dtype=np.float32))
    edge_index = np.asarray(inputs["edge_index"])
    W1 = np.ascontiguousarray(np.asarray(inputs["W1"], dtype=np.float32))
    a_src1 = np.asarray(inputs["a_src1"], dtype=np.float32)
    a_dst1 = np.asarray(inputs["a_dst1"], dtype=np.float32)
    b1 = np.asarray(inputs["b1"], dtype=np.float32)
    W2 = np.ascontiguousarray(np.asarray(inputs["W2"], dtype=np.float32))
    a_src2 = np.asarray(inputs["a_src2"], dtype=np.float32)
    a_dst2 = np.asarray(inputs["a_dst2"], dtype=np.float32)
    b2 = np.asarray(inputs["b2"], dtype=np.float32)

    debug = bool(int(os.environ.get("GAT_DEBUG", "0")))

    T0, T1, tot, idx_np, dstcol_np, dstrow_np = _preprocess(edge_index)
    key = (tuple(T0), tuple(T1), tot, debug)
    if key not in _CACHE:
        _CACHE[key] = _build(T0, T1, tot, debug=debug)
    nc = _CACHE[key]

    def bcast(a, d):
        return np.ascontiguousarray(np.broadcast_to(a.reshape(1, d), (128, d)))

    import ml_dtypes
    common = {
        "W1": W1, "W2": W2,
        "asrc1b": bcast(a_src1, D1), "adst1b": bcast(a_dst1, D1),
        "asrc2b": bcast(a_src2, OUT_DIM), "adst2b": bcast(a_dst2, OUT_DIM),
        "b1b": bcast(b1, D1), "b2b": bcast(b2, OUT_DIM),
        "iota_row": np.ascontiguousarray(
            np.broadcast_to(np.arange(128, dtype=np.float32)[None, :], (128, 128))),
        "iota_col": np.arange(128, dtype=np.float32)[:, None].copy(),
        "ones_row": np.ones((1, 128), ml_dtypes.bfloat16),
        "ident": np.eye(128, dtype=np.float32),
    }
    in_maps = []
    for c in range(NCORES):
        m = dict(common)
        m["xT"] = np.ascontiguousarray(x[c * NPC:(c + 1) * NPC, :].T)
        m["idx"] = idx_np[c]
        m["dstcol"] = np.ascontiguousarray(dstcol_np[c])
        m["dstrow"] = np.ascontiguousarray(dstrow_np[c]).astype(ml_dtypes.bfloat16)
        in_maps.append(m)

    res = run_bass_kernel_spmd(nc, in_maps, core_ids=list(range(NCORES)),
                               trace_cores=[0])
    LAST_EXEC_NS = res.exec_time_ns
    kernel.last_results = res

    return np.concatenate([res.results[c]["out"] for c in range(NCORES)], axis=0)
